# revision 9
# baseline (speedup 1.0000x reference)
"""FFM layer on 8 Trainium2 NeuronCores — conv-hybrid, T-block sharded.

Each core owns a 512-row block of the sequence and produces its block of
the output directly; the only collective is a 3 KB AllGather of scan
carries (fully overlapped with the conv matmuls).

  zm[t,o] = sum_{i,Delta} z[t-Delta, i] * G[(i,Delta), o]
  G[(i,D),o] = rho_i^D * sum_j cos(b_j D) Wre[i,j,o] + sin(b_j D) Wim[i,j,o]

Traces 6..63 (kernel decays within <= 128 steps) go through this causal-
conv-as-matmul with per-trace truncation; traces 0..5 (slow decay) use
three 128-partition tiles of rotated real-scan pairs (C,S) over the local
block plus a carry correction C' = C + rho^{tau+1} * I_c, where I_c is a
weighted sum of the other cores' block-end columns E. E is computed
directly as a weighted reduction of the scan INPUT (accum_out of a fused
multiply), so the collective launches before the scans even finish.

Schedule: PE warms up on dummy matmuls (p-state ramp), sync queue carries
the latency-critical loads in dependency-chained order, the ACT HWDGE
queue streams im2col (diagonal DRAM->SBUF views of z) interleaved with
the G table, and each psum's LayerNorm runs right after its stop matmul.
"""

import numpy as np
from contextlib import ExitStack

import concourse.bacc as bacc
import concourse.bass as bass
import concourse.tile as tile
from concourse import mybir
from concourse.bass_utils import run_bass_kernel_spmd

T, IN, TR, CTX, OUT = 4096, 512, 64, 64, 512
NCORES = 8
BLK = T // NCORES       # 512 rows per core
NTILE = 3               # scan tiles (2 traces each)
NSCAN = 2 * NTILE       # traces handled by scan
LN_EPS = 1e-6
FP32 = mybir.dt.float32
BF16 = mybir.dt.bfloat16
AOT = mybir.AluOpType
AFT = mybir.ActivationFunctionType

# conv plan: per-trace entries (trace, L) with L multiple of 128, then
# packed classes (first_trace, n_traces, L) with 128//L traces per chunk.
PER_TRACE = [(6, 128), (7, 128), (8, 128), (9, 128)]
PACKED = [(10, 12, 64), (22, 24, 32), (46, 18, 16)]

# mcol column layout
M_RHO = 0                       # NTILE cols: rho per tile
M_WGT = NTILE                   # 16*NTILE: carry weights (b, tile, fld)
M_S0 = M_WGT + 16 * NTILE       # 2*NTILE: state0 carry term
M_MASK = M_S0 + 2 * NTILE       # 1: prev-block mask
M_BIAS = M_MASK + 1             # 1: b_pre|b_gin
M_NCOL = M_BIAS + 1

_CACHE: dict = {}


def _conv_plan():
    """entries for DMA generation + flat row map [(trace, delta)], -1=pad."""
    entries = []
    rowmap = []
    c0 = 0
    for i, L in PER_TRACE:
        k = L // 128
        entries.append(("per_trace", i, L, c0, k))
        block = np.full((k * 128, 2), (-1, 0), np.int64)
        for cc in range(k):
            for p in range(128):
                dp = p * k + cc
                block[cc * 128 + p] = (i, L - 1 - dp)
        rowmap.append(block)
        c0 += k
    for i0, nt, L in PACKED:
        tpc = 128 // L
        nch = (nt + tpc - 1) // tpc
        entries.append(("packed", i0, nt, L, c0, nch))
        block = np.full((nch * 128, 2), (-1, 0), np.int64)
        for cc in range(nch):
            for h in range(tpc):
                tr = i0 + tpc * cc + h
                if tr >= i0 + nt:
                    continue
                for dpr in range(L):
                    block[cc * 128 + h * L + dpr] = (tr, L - 1 - dpr)
        rowmap.append(block)
        c0 += nch
    return entries, np.concatenate(rowmap), c0


CONV_ENTRIES, ROWMAP, NCH = _conv_plan()
KCONV = NCH * 128


def _ap(t: bass.AP, col_off: int, dims) -> bass.AP:
    """AP over an SBUF tile slice: keep its partition dim, custom free dims."""
    return bass.AP(tensor=t.tensor, offset=t.offset + col_off,
                   ap=[t.ap[0]] + list(dims))


def _free_bcast(col: bass.AP, n: int) -> bass.AP:
    return bass.AP(tensor=col.tensor, offset=col.offset,
                   ap=[col.ap[0], [0, n]])


def _chain(insts):
    """Order-chain a list of instructions (scheduler hint, no semaphores)."""
    for a, b in zip(insts[1:], insts[:-1]):
        if a is not None and b is not None:
            bass._add_dep_helper(a.ins, b.ins, False, "dma priority chain")


def _build_module(with_state0: bool = False):
    nc = bacc.Bacc("TRN2", target_bir_lowering=False, debug=False,
                   num_devices=NCORES)

    def inp(name, shape, dt):
        return nc.dram_tensor(name, list(shape), dt, kind="ExternalInput").ap()

    xT_in = inp("xT_in", (IN, 2 * BLK), BF16)        # [prev block | own block]^T
    wpg = inp("wpg", (IN, 128), BF16)                # [W_pre | W_gin] columns
    trig = inp("trig", (128, 2 * BLK), BF16)         # cos | sin (global t)
    rp = inp("rp", (128, 2 * NTILE * BLK), BF16)     # rhoprev | rhopow per tile
    mcol = inp("mcol", (128, M_NCOL), FP32)
    wmix_sc = inp("wmix_sc", (NTILE * 2 * 128, OUT), BF16)
    wgs = inp("wgs", (8 * 128, OUT), BF16)           # gout 4 chunks | skip 4
    gtab = inp("gtab", (KCONV, OUT), BF16)           # conv kernel table
    ones_row = inp("ones_row", (1, 128), BF16)
    brow = inp("brow", (1, 3 * OUT), BF16)           # bgout | bskip | bmix

    outc = nc.dram_tensor("outc", [BLK, OUT], FP32, kind="ExternalOutput").ap()
    groups = [list(range(NCORES))]

    with tile.TileContext(nc) as tc, ExitStack() as ctx:
        const = ctx.enter_context(tc.tile_pool(name="const", bufs=1))
        dram = ctx.enter_context(tc.tile_pool(name="dram", bufs=1, space="DRAM"))

        # ---- latency-ordered loads (sync queue, dep-chained) -------------
        ld = []
        wpg_sb = const.tile([128, 4 * 128], BF16)
        ld.append(nc.sync.dma_start(
            wpg_sb, bass.AP(tensor=wpg.tensor, offset=0,
                            ap=[[128, 128], [128 * 128, 4], [1, 128]])))
        xt = const.tile([128, 4 * 2 * BLK], BF16)    # (IN-chunk, [prev|own] t)
        for h in (1, 0):                             # own half first
            ld.append(nc.sync.dma_start(
                _ap(xt, h * BLK, [[2 * BLK, 4], [1, BLK]]),
                bass.AP(tensor=xT_in.tensor, offset=h * BLK,
                        ap=[[2 * BLK, 128], [128 * 2 * BLK, 4], [1, BLK]])))
        trig_sb = const.tile([128, 2 * BLK], BF16)
        tld = nc.sync.dma_start(trig_sb, trig)
        ld.insert(2, tld)    # trig may land before xt-prev
        cosb = trig_sb[:, 0:BLK]
        sinb = trig_sb[:, BLK:2 * BLK]
        mcol_sb = const.tile([128, M_NCOL], FP32)
        ld.append(nc.sync.dma_start(mcol_sb, mcol))
        rp_sb = const.tile([128, 2 * NTILE * BLK], BF16)
        ld.append(nc.sync.dma_start(rp_sb, rp))

        def rprev(t):
            return rp_sb[:, (2 * t) * BLK:(2 * t + 1) * BLK]

        def rpow(t):
            return rp_sb[:, (2 * t + 1) * BLK:(2 * t + 2) * BLK]

        wmix_sb = const.tile([128, NTILE * 2 * OUT], BF16)
        ld.append(nc.sync.dma_start(
            wmix_sb, bass.AP(tensor=wmix_sc.tensor, offset=0,
                             ap=[[OUT, 128], [128 * OUT, NTILE * 2], [1, OUT]])))
        wgs_sb = const.tile([128, 8 * OUT], BF16)
        ld.append(nc.sync.dma_start(
            wgs_sb, bass.AP(tensor=wgs.tensor, offset=0,
                            ap=[[OUT, 128], [128 * OUT, 8], [1, OUT]])))
        ones_sb = const.tile([1, 128], BF16)
        ld.append(nc.sync.dma_start(ones_sb, ones_row))
        brow_sb = const.tile([1, 3 * OUT], BF16)
        ld.append(nc.sync.dma_start(brow_sb, brow))
        _chain(ld)
        eps_sb = const.tile([128, 1], FP32)
        nc.vector.memset(eps_sb, LN_EPS)

        zD = dram.tile([TR, 2 * BLK], BF16, name="zD")
        E_my = dram.tile([128, 2 * NTILE], FP32, name="E_my")
        E_all = dram.tile([128 * NCORES, 2 * NTILE], FP32, name="E_all")

        # ---- PE warmup: p-state ramp on dummy matmuls --------------------
        warm = const.tile([128, BLK], BF16)
        nc.vector.memset(warm, 0.0)
        with tc.tile_pool(name="psw", bufs=1, space="PSUM") as psw:
            wps = psw.tile([128, BLK], FP32, tag="warm")
            for i in range(8):
                nc.tensor.matmul(wps, warm[:, 0:128], warm,
                                 start=(i == 0), stop=(i == 7))

        # ---- A: gated z, own block first (feeds the scan/E chain) --------
        zbs = [const.tile([128, BLK], BF16, tag=f"zb{t}", name=f"zb{t}")
               for t in range(NTILE)]
        with tc.tile_pool(name="psa", bufs=2, space="PSUM") as psa:
            for h in (1, 0):
                ps = psa.tile([128, BLK], FP32, tag="za", bufs=2)
                for ki in range(4):
                    nc.tensor.matmul(
                        ps, wpg_sb[:, ki * 128:(ki + 1) * 128],
                        xt[:, ki * 2 * BLK + h * BLK: ki * 2 * BLK + (h + 1) * BLK],
                        start=(ki == 0), stop=(ki == 3))
                pre_sb = const.tile([64, BLK], FP32, tag=f"pre{h}")
                nc.scalar.activation(pre_sb, ps[0:64, :], AFT.Identity,
                                     bias=mcol_sb[0:64, M_BIAS:M_BIAS + 1])
                sig_sb = const.tile([64, BLK], FP32, tag=f"sig{h}")
                nc.scalar.activation(sig_sb, ps[64:128, :], AFT.Sigmoid,
                                     bias=mcol_sb[64:128, M_BIAS:M_BIAS + 1])
                zt = const.tile([64, BLK], BF16, tag=f"z{h}")
                if h == 0:   # prev block: masked to 0 on core 0
                    nc.vector.scalar_tensor_tensor(
                        zt, pre_sb, mcol_sb[0:64, M_MASK:M_MASK + 1], sig_sb,
                        op0=AOT.mult, op1=AOT.mult)
                else:
                    nc.vector.tensor_mul(zt, pre_sb, sig_sb)
                nc.sync.dma_start(
                    bass.AP(tensor=zD.tensor, offset=zD.offset + h * BLK,
                            ap=[[2 * BLK, TR], [1, BLK]]), zt)
                if h == 1:   # broadcast own-block z for the scan traces now
                    for t in range(NTILE):
                        for il in range(2):
                            nc.sync.dma_start(
                                zbs[t][il * CTX:(il + 1) * CTX, :],
                                bass.AP(tensor=zD.tensor,
                                        offset=(zD.offset
                                                + (2 * t + il) * 2 * BLK + BLK),
                                        ap=[[0, CTX], [1, BLK]]))

        # ---- scan tiles: inputs + E columns first, then the scans --------
        cc_ts, ss_ts, C_ts, S_ts = [], [], [], []
        E_sb = const.tile([128, 2 * NTILE], FP32)
        for t in range(NTILE):
            cc_t = const.tile([128, BLK], BF16, tag=f"cc{t}")
            nc.vector.tensor_mul(cc_t, zbs[t], cosb)
            ss_t = const.tile([128, BLK], BF16, tag=f"ss{t}")
            nc.vector.tensor_mul(ss_t, zbs[t], sinb)
            scr = const.tile([128, BLK], BF16, tag="scr", bufs=2)
            nc.vector.scalar_tensor_tensor(
                scr, rprev(t), 1.0, cc_t, op0=AOT.mult, op1=AOT.mult,
                accum_out=E_sb[:, 2 * t:2 * t + 1])
            scr2 = const.tile([128, BLK], BF16, tag="scr", bufs=2)
            nc.vector.scalar_tensor_tensor(
                scr2, rprev(t), 1.0, ss_t, op0=AOT.mult, op1=AOT.mult,
                accum_out=E_sb[:, 2 * t + 1:2 * t + 2])
            cc_ts.append(cc_t)
            ss_ts.append(ss_t)

        # E exchange entirely on the gpsimd queue (no head-of-line blocking)
        nc.gpsimd.dma_start(E_my, E_sb)
        nc.gpsimd.collective_compute(
            "AllGather", AOT.bypass, replica_groups=groups,
            ins=[E_my.opt()], outs=[E_all.opt()])
        E_all_sb = const.tile([128, 16 * NTILE], FP32)
        nc.gpsimd.dma_start(
            E_all_sb,
            bass.AP(tensor=E_all.tensor, offset=E_all.offset,
                    ap=[[2 * NTILE, 128], [256 * NTILE, NCORES],
                        [1, 2 * NTILE]]))

        for t in range(NTILE):
            C_t = const.tile([128, BLK], BF16, tag=f"C{t}")
            nc.vector.tensor_tensor_scan(
                C_t, _free_bcast(mcol_sb[:, M_RHO + t:M_RHO + t + 1], BLK),
                cc_ts[t], initial=0.0, op0=AOT.mult, op1=AOT.add)
            S_t = const.tile([128, BLK], BF16, tag=f"S{t}")
            nc.vector.tensor_tensor_scan(
                S_t, _free_bcast(mcol_sb[:, M_RHO + t:M_RHO + t + 1], BLK),
                ss_ts[t], initial=0.0, op0=AOT.mult, op1=AOT.add)
            C_ts.append(C_t)
            S_ts.append(S_t)

        # ---- im2col + G table, interleaved on the ACT HWDGE queue --------
        imcol = const.tile([128, NCH * BLK], BF16)
        g_sb = const.tile([128, NCH * OUT], BF16)
        nq = (NCH + 3) // 4
        gq = [0, nq, 2 * nq, 3 * nq, NCH]
        stream = []

        def load_g(q):
            h0, nh = gq[q], gq[q + 1] - gq[q]
            stream.append(nc.scalar.dma_start(
                _ap(g_sb, h0 * OUT, [[OUT, nh], [1, OUT]]),
                bass.AP(tensor=gtab.tensor, offset=h0 * 128 * OUT,
                        ap=[[OUT, 128], [128 * OUT, nh], [1, OUT]])))

        gq_next = 0

        def maybe_g(c_done):
            nonlocal gq_next
            while gq_next < 4 and gq[gq_next] <= c_done:
                load_g(gq_next)
                gq_next += 1

        maybe_g(0)
        for e in CONV_ENTRIES:
            if e[0] == "per_trace":
                _, i, L, c0, k = e
                stream.append(nc.scalar.dma_start(
                    _ap(imcol, c0 * BLK, [[BLK, k], [1, BLK]]),
                    bass.AP(tensor=zD.tensor,
                            offset=zD.offset + i * 2 * BLK + BLK + 1 - L,
                            ap=[[k, 128], [1, k], [1, BLK]])))
                maybe_g(c0 + k)
            else:
                _, i0, nt, L, c0, nch = e
                tpc = 128 // L
                for h in range(tpc):
                    nch_h = (nt - h + tpc - 1) // tpc
                    base = imcol[h * L:(h + 1) * L, :]
                    stream.append(nc.scalar.dma_start(
                        bass.AP(tensor=base.tensor,
                                offset=base.offset + c0 * BLK,
                                ap=[base.ap[0], [BLK, nch_h], [1, BLK]]),
                        bass.AP(tensor=zD.tensor,
                                offset=(zD.offset + (i0 + h) * 2 * BLK
                                        + BLK + 1 - L),
                                ap=[[1, L], [tpc * 2 * BLK, nch_h], [1, BLK]])))
                npad = nch * 128 - ((nt - 1) // tpc) * 128 - \
                    ((nt - 1) % tpc + 1) * L
                if npad > 0:   # ragged tail: fill with dup rows (G=0)
                    base = imcol[128 - npad:128, :]
                    stream.append(nc.scalar.dma_start(
                        bass.AP(tensor=base.tensor,
                                offset=base.offset + (c0 + nch - 1) * BLK,
                                ap=[base.ap[0], [1, BLK]]),
                        bass.AP(tensor=zD.tensor,
                                offset=zD.offset + (TR - 1) * 2 * BLK + BLK,
                                ap=[[0, npad], [1, BLK]])))
                maybe_g(c0 + nch)
        _chain([ld[-1]] + stream)

        # ---- carry correction + rotate-back (DVE, after AllGather) -------
        prod = const.tile([128, 16 * NTILE], FP32)
        nc.vector.tensor_mul(prod, E_all_sb, mcol_sb[:, M_WGT:M_WGT + 16 * NTILE])
        w8 = 8 * NTILE
        f1 = const.tile([128, w8], FP32)
        nc.vector.tensor_add(f1, prod[:, 0:w8], prod[:, w8:2 * w8])
        f2 = const.tile([128, w8 // 2], FP32)
        nc.vector.tensor_add(f2, f1[:, 0:w8 // 2], f1[:, w8 // 2:w8])
        icis = const.tile([128, 2 * NTILE], FP32)
        if with_state0:
            f3 = const.tile([128, 2 * NTILE], FP32)
            nc.vector.tensor_add(f3, f2[:, 0:2 * NTILE], f2[:, 2 * NTILE:])
            nc.vector.tensor_add(icis, f3, mcol_sb[:, M_S0:M_S0 + 2 * NTILE])
        else:
            nc.vector.tensor_add(icis, f2[:, 0:2 * NTILE], f2[:, 2 * NTILE:])

        s_rs, s_is = [], []
        for t in range(NTILE):
            Cc = const.tile([128, BLK], BF16, tag=f"Cc{t}")
            nc.vector.scalar_tensor_tensor(
                Cc, rpow(t), icis[:, 2 * t:2 * t + 1], C_ts[t],
                op0=AOT.mult, op1=AOT.add)
            Sc = const.tile([128, BLK], BF16, tag=f"Sc{t}")
            nc.vector.scalar_tensor_tensor(
                Sc, rpow(t), icis[:, 2 * t + 1:2 * t + 2], S_ts[t],
                op0=AOT.mult, op1=AOT.add)
            m1 = const.tile([128, BLK], BF16, tag="m1", bufs=2)
            nc.vector.tensor_mul(m1, Cc, cosb)
            m2 = const.tile([128, BLK], BF16, tag="m2", bufs=2)
            nc.vector.tensor_mul(m2, Sc, sinb)
            s_r = const.tile([128, BLK], BF16, tag=f"sr{t}")
            nc.vector.tensor_add(s_r, m1, m2)
            m3 = const.tile([128, BLK], BF16, tag="m3", bufs=2)
            nc.vector.tensor_mul(m3, Cc, sinb)
            m4 = const.tile([128, BLK], BF16, tag="m4", bufs=2)
            nc.vector.tensor_mul(m4, Sc, cosb)
            s_i = const.tile([128, BLK], BF16, tag=f"si{t}")
            nc.vector.tensor_sub(s_i, m3, m4)
            s_rs.append(s_r)
            s_is.append(s_i)

        # ---- PE: conv sweeps + B-prep; stops + LayerNorm per psum --------
        gout_st = const.tile([128, 4 * OUT], BF16)
        skip_st = const.tile([128, 4 * OUT], BF16)
        t2_st = const.tile([128, 4 * OUT], BF16)

        with tc.tile_pool(name="psz", bufs=1, space="PSUM") as psz, \
                tc.tile_pool(name="psb", bufs=2, space="PSUM") as psb, \
                tc.tile_pool(name="pb", bufs=2) as pb:
            zmps = [psz.tile([128, OUT], FP32, tag=f"zm{i}", name=f"zm{i}")
                    for i in range(4)]

            def bprep(tc4):
                toff = 512 + tc4 * 128
                osl = slice(tc4 * OUT, (tc4 + 1) * OUT)
                ps_go = psb.tile([128, OUT], FP32, tag="go", bufs=2)
                for ki in range(4):
                    nc.tensor.matmul(
                        ps_go,
                        xt[:, ki * 2 * BLK + toff: ki * 2 * BLK + toff + 128],
                        wgs_sb[:, ki * OUT:(ki + 1) * OUT],
                        start=(ki == 0), stop=False)
                nc.tensor.matmul(ps_go, ones_sb, brow_sb[:, 0:OUT],
                                 start=False, stop=True)
                nc.scalar.activation(gout_st[:, osl], ps_go, AFT.Sigmoid)
                ps_sk = psb.tile([128, OUT], FP32, tag="sk", bufs=2)
                for ki in range(4):
                    nc.tensor.matmul(
                        ps_sk,
                        xt[:, ki * 2 * BLK + toff: ki * 2 * BLK + toff + 128],
                        wgs_sb[:, (4 + ki) * OUT:(5 + ki) * OUT],
                        start=(ki == 0), stop=False)
                nc.tensor.matmul(ps_sk, ones_sb, brow_sb[:, OUT:2 * OUT],
                                 start=False, stop=True)
                nc.scalar.copy(skip_st[:, osl], ps_sk)
                # t2 = (gout-1)*skip precomputed off the critical B path
                nc.gpsimd.scalar_tensor_tensor(
                    t2_st[:, osl], gout_st[:, osl], 1.0, skip_st[:, osl],
                    op0=AOT.subtract, op1=AOT.mult)

            def bphase(tc4):
                osl = slice(tc4 * OUT, (tc4 + 1) * OUT)
                v = pb.tile([128, OUT], BF16, tag="v")
                nc.vector.tensor_mul(v, zmps[tc4], gout_st[:, osl])
                stats = pb.tile([128, 6], FP32, tag="stats")
                nc.vector.bn_stats(stats, v)
                mv = pb.tile([128, 2], FP32, tag="mv")
                nc.vector.bn_aggr(mv, stats)
                sd = pb.tile([128, 1], FP32, tag="sd")
                nc.scalar.activation(sd, mv[:, 1:2], AFT.Sqrt, bias=eps_sb)
                rstd = pb.tile([128, 1], FP32, tag="rstd")
                nc.vector.reciprocal(rstd, sd)
                ln = pb.tile([128, OUT], BF16, tag="ln")
                nc.vector.tensor_scalar(
                    ln, v, mv[:, 0:1], rstd, op0=AOT.subtract, op1=AOT.mult)
                res = pb.tile([128, OUT], FP32, tag="res")
                nc.vector.tensor_sub(res, ln, t2_st[:, osl])
                nc.gpsimd.dma_start(outc[tc4 * 128:(tc4 + 1) * 128, :], res)

            # conv sweeps; B-prep fills the DMA-paced first sweep
            for tc4 in range(4):
                for c in range(NCH):
                    nc.tensor.matmul(
                        zmps[tc4],
                        imcol[:, c * BLK + tc4 * 128: c * BLK + tc4 * 128 + 128],
                        g_sb[:, c * OUT:(c + 1) * OUT],
                        start=(c == 0), stop=False)
                    if tc4 == 0 and c in (2, 6, 10, 14):
                        bprep((2, 6, 10, 14).index(c))
            # stops + LayerNorm per psum
            for tc4 in range(4):
                nc.tensor.matmul(zmps[tc4], ones_sb,
                                 brow_sb[:, 2 * OUT:3 * OUT],
                                 start=False, stop=False)
                for t in range(NTILE):
                    nc.tensor.matmul(
                        zmps[tc4], s_rs[t][:, tc4 * 128:(tc4 + 1) * 128],
                        wmix_sb[:, (2 * t) * OUT:(2 * t + 1) * OUT],
                        start=False, stop=False)
                    nc.tensor.matmul(
                        zmps[tc4], s_is[t][:, tc4 * 128:(tc4 + 1) * 128],
                        wmix_sb[:, (2 * t + 1) * OUT:(2 * t + 2) * OUT],
                        start=False, stop=(t == NTILE - 1))
                bphase(tc4)

    nc.compile()
    return nc


def _prep_inputs(inputs):
    x = np.asarray(inputs["x"], np.float32)
    state0 = np.asarray(inputs["state0"], np.float64)
    a = np.abs(np.asarray(inputs["ffa_a"], np.float64))
    b = np.asarray(inputs["ffa_b"], np.float64)
    rho = np.exp(-a)
    W_pre = np.asarray(inputs["W_pre"], np.float32)
    b_pre = np.asarray(inputs["b_pre"], np.float32)
    W_gin = np.asarray(inputs["W_gin"], np.float32)
    b_gin = np.asarray(inputs["b_gin"], np.float32)
    W_gout = np.asarray(inputs["W_gout"], np.float32)
    b_gout = np.asarray(inputs["b_gout"], np.float32)
    W_skip = np.asarray(inputs["W_skip"], np.float32)
    b_skip = np.asarray(inputs["b_skip"], np.float32)
    W_mix = np.asarray(inputs["W_mix"], np.float64)
    b_mix = np.asarray(inputs["b_mix"], np.float32)
    Wm = W_mix.reshape(TR, 2, CTX, OUT)

    bf16 = mybir.dt.np(BF16)

    # G table (same for all cores)
    G = np.zeros((KCONV, OUT), np.float32)
    for i in range(NSCAN, TR):
        rows = np.nonzero(ROWMAP[:, 0] == i)[0]
        if len(rows) == 0:
            continue
        ds = ROWMAP[rows, 1].astype(np.float64)
        ang = np.outer(ds, b)
        G[rows] = ((np.cos(ang) @ Wm[i, 0] + np.sin(ang) @ Wm[i, 1])
                   * (rho[i] ** ds)[:, None]).astype(np.float32)
    G = G.astype(bf16)

    wpg_h = np.concatenate([W_pre, W_gin], axis=1).astype(bf16)   # (512,128)
    wgs_h = np.concatenate([W_gout.reshape(4, 128, OUT),
                            W_skip.reshape(4, 128, OUT)], axis=0) \
        .reshape(8 * 128, OUT).astype(bf16)
    wmix_h = np.stack(
        [Wm[2 * t + il, fld]
         for t in range(NTILE) for fld in range(2) for il in range(2)]) \
        .reshape(NTILE * 2, 128, OUT).reshape(NTILE * 2 * 128, OUT).astype(bf16)
    ones_h = np.ones((1, 128), bf16)
    brow_h = np.concatenate([b_gout, b_skip, b_mix])[None, :].astype(bf16)

    jj = np.tile(np.arange(CTX), 2)                 # j per partition
    tau = np.arange(BLK, dtype=np.float64)
    rp_h = np.zeros((128, 2 * NTILE * BLK))
    for t in range(NTILE):
        ii = np.repeat([2 * t, 2 * t + 1], CTX)
        rp_h[:, 2 * t * BLK:(2 * t + 1) * BLK] = \
            rho[ii][:, None] ** (BLK - 1.0 - tau[None, :])
        rp_h[:, (2 * t + 1) * BLK:(2 * t + 2) * BLK] = \
            rho[ii][:, None] ** (tau[None, :] + 1.0)
    rp_h = rp_h.astype(bf16)

    s0c = state0[0, :, :, 0] + 1j * state0[0, :, :, 1]   # (TR, CTX)
    r_init = np.exp(1j * b)[None, :] * s0c[0:NSCAN]      # R_{-1} per (i,j)
    initC = r_init.real.reshape(NSCAN, CTX)
    initS = (-r_init.imag).reshape(NSCAN, CTX)

    xb = x.astype(bf16)
    in_maps = []
    for c in range(NCORES):
        t0 = c * BLK
        xT_h = np.zeros((IN, 2 * BLK), bf16)
        if c > 0:
            xT_h[:, 0:BLK] = xb[t0 - BLK:t0].T
        xT_h[:, BLK:] = xb[t0:t0 + BLK].T

        tg = (t0 + np.arange(BLK, dtype=np.float64))[None, :]
        ang = b[jj][:, None] * tg                    # (128, BLK)
        trig_h = np.concatenate([np.cos(ang), np.sin(ang)], axis=1).astype(bf16)

        mcol_h = np.zeros((128, M_NCOL), np.float32)
        for t in range(NTILE):
            ii = np.repeat([2 * t, 2 * t + 1], CTX)
            mcol_h[:, M_RHO + t] = rho[ii]
            for bb in range(c):
                w = rho[ii] ** (512.0 * (c - 1 - bb))
                mcol_h[:, M_WGT + bb * 2 * NTILE + 2 * t] = w
                mcol_h[:, M_WGT + bb * 2 * NTILE + 2 * t + 1] = w
            rr = rho[ii] ** (512.0 * c)
            mcol_h[:, M_S0 + 2 * t] = rr * np.concatenate(
                [initC[2 * t], initC[2 * t + 1]])
            mcol_h[:, M_S0 + 2 * t + 1] = rr * np.concatenate(
                [initS[2 * t], initS[2 * t + 1]])
        mcol_h[0:64, M_MASK] = 0.0 if c == 0 else 1.0
        mcol_h[0:64, M_BIAS] = b_pre
        mcol_h[64:128, M_BIAS] = b_gin

        in_maps.append({
            "xT_in": xT_h,
            "wpg": wpg_h,
            "trig": trig_h,
            "rp": rp_h,
            "mcol": mcol_h,
            "wmix_sc": wmix_h,
            "wgs": wgs_h,
            "gtab": G,
            "ones_row": ones_h,
            "brow": brow_h,
        })
    return in_maps


def _assemble(results) -> np.ndarray:
    return np.concatenate(
        [np.asarray(results[c]["outc"]) for c in range(NCORES)], axis=0)


def _get_module(with_state0: bool = False):
    key = f"m{int(with_state0)}"
    if key not in _CACHE:
        _CACHE[key] = _build_module(with_state0)
    return _CACHE[key]


def kernel(**inputs) -> np.ndarray:
    with_s0 = bool(np.any(np.asarray(inputs["state0"])))
    nc = _get_module(with_s0)
    in_maps = _prep_inputs(inputs)
    res = run_bass_kernel_spmd(nc, in_maps, list(range(NCORES)))
    return _assemble(res.results)


if __name__ == "__main__":
    import reference
    inputs = reference.setup_inputs()
    out = kernel(**{k: np.asarray(v) for k, v in inputs.items()})
    print("kernel output", out.shape, out.dtype)


# revision 10
# speedup vs baseline: 1.0165x; 1.0165x over previous
"""FFM layer on 8 Trainium2 NeuronCores — conv-hybrid, T-block sharded.

Each core owns a 512-row block of the sequence and produces its block of
the output directly; the only collective is a 3 KB AllGather of scan
carries (fully overlapped with the conv matmuls).

  zm[t,o] = sum_{i,Delta} z[t-Delta, i] * G[(i,Delta), o]
  G[(i,D),o] = rho_i^D * sum_j cos(b_j D) Wre[i,j,o] + sin(b_j D) Wim[i,j,o]

Traces 6..63 (kernel decays within <= 128 steps) go through this causal-
conv-as-matmul with per-trace truncation; traces 0..5 (slow decay) use
three 128-partition tiles of rotated real-scan pairs (C,S) over the local
block plus a carry correction C' = C + rho^{tau+1} * I_c, where I_c is a
weighted sum of the other cores' block-end columns E. E is computed
directly as a weighted reduction of the scan INPUT (accum_out of a fused
multiply), so the collective launches before the scans even finish.

Schedule: PE warms up on dummy matmuls (p-state ramp), sync queue carries
the latency-critical loads in dependency-chained order, the ACT HWDGE
queue streams im2col (diagonal DRAM->SBUF views of z) interleaved with
the G table, and each psum's LayerNorm runs right after its stop matmul.
"""

import numpy as np
from contextlib import ExitStack

import concourse.bacc as bacc
import concourse.bass as bass
import concourse.tile as tile
from concourse import mybir
from concourse.bass_utils import run_bass_kernel_spmd

T, IN, TR, CTX, OUT = 4096, 512, 64, 64, 512
NCORES = 8
BLK = T // NCORES       # 512 rows per core
NTILE = 3               # scan tiles (2 traces each)
NSCAN = 2 * NTILE       # traces handled by scan
LN_EPS = 1e-6
FP32 = mybir.dt.float32
BF16 = mybir.dt.bfloat16
AOT = mybir.AluOpType
AFT = mybir.ActivationFunctionType

# conv plan: per-trace entries (trace, L) with L multiple of 128, then
# packed classes (first_trace, n_traces, L) with 128//L traces per chunk.
PER_TRACE = [(6, 128), (7, 128), (8, 128), (9, 128)]
PACKED = [(10, 12, 64), (22, 24, 32), (46, 18, 16)]

# mcol column layout
M_RHO = 0                       # NTILE cols: rho per tile
M_WGT = NTILE                   # 16*NTILE: carry weights (b, tile, fld)
M_S0 = M_WGT + 16 * NTILE       # 2*NTILE: state0 carry term
M_MASK = M_S0 + 2 * NTILE       # 1: prev-block mask
M_BIAS = M_MASK + 1             # 1: b_pre|b_gin
M_NCOL = M_BIAS + 1

_CACHE: dict = {}


def _conv_plan():
    """entries for DMA generation + flat row map [(trace, delta)], -1=pad."""
    entries = []
    rowmap = []
    c0 = 0
    for i, L in PER_TRACE:
        k = L // 128
        entries.append(("per_trace", i, L, c0, k))
        block = np.full((k * 128, 2), (-1, 0), np.int64)
        for cc in range(k):
            for p in range(128):
                dp = p * k + cc
                block[cc * 128 + p] = (i, L - 1 - dp)
        rowmap.append(block)
        c0 += k
    for i0, nt, L in PACKED:
        tpc = 128 // L
        nch = (nt + tpc - 1) // tpc
        entries.append(("packed", i0, nt, L, c0, nch))
        block = np.full((nch * 128, 2), (-1, 0), np.int64)
        for cc in range(nch):
            for h in range(tpc):
                tr = i0 + tpc * cc + h
                if tr >= i0 + nt:
                    continue
                for dpr in range(L):
                    block[cc * 128 + h * L + dpr] = (tr, L - 1 - dpr)
        rowmap.append(block)
        c0 += nch
    return entries, np.concatenate(rowmap), c0


CONV_ENTRIES, ROWMAP, NCH = _conv_plan()
KCONV = NCH * 128


def _ap(t: bass.AP, col_off: int, dims) -> bass.AP:
    """AP over an SBUF tile slice: keep its partition dim, custom free dims."""
    return bass.AP(tensor=t.tensor, offset=t.offset + col_off,
                   ap=[t.ap[0]] + list(dims))


def _free_bcast(col: bass.AP, n: int) -> bass.AP:
    return bass.AP(tensor=col.tensor, offset=col.offset,
                   ap=[col.ap[0], [0, n]])


def _chain(insts):
    """Order-chain a list of instructions (scheduler hint, no semaphores)."""
    for a, b in zip(insts[1:], insts[:-1]):
        if a is not None and b is not None:
            bass._add_dep_helper(a.ins, b.ins, False, "dma priority chain")


def _build_module(with_state0: bool = False):
    nc = bacc.Bacc("TRN2", target_bir_lowering=False, debug=False,
                   num_devices=NCORES)

    def inp(name, shape, dt):
        return nc.dram_tensor(name, list(shape), dt, kind="ExternalInput").ap()

    xT_in = inp("xT_in", (IN, 2 * BLK), BF16)        # [prev block | own block]^T
    wpg = inp("wpg", (IN, 128), BF16)                # [W_pre | W_gin] columns
    trig = inp("trig", (128, 2 * BLK), BF16)         # cos | sin (global t)
    rpv = inp("rpv", (128, NTILE * BLK), BF16)       # rhoprev per tile
    rpw = inp("rpw", (128, NTILE * BLK), BF16)       # rhopow per tile
    mcol = inp("mcol", (128, M_NCOL), FP32)
    wmix_sc = inp("wmix_sc", (NTILE * 2 * 128, OUT), BF16)
    wgs = inp("wgs", (8 * 128, OUT), BF16)           # gout 4 chunks | skip 4
    gtab = inp("gtab", (KCONV, OUT), BF16)           # conv kernel table
    ones_row = inp("ones_row", (1, 128), BF16)
    brow = inp("brow", (1, 3 * OUT), BF16)           # bgout | bskip | bmix

    outc = nc.dram_tensor("outc", [BLK, OUT], FP32, kind="ExternalOutput").ap()
    groups = [list(range(NCORES))]

    with tile.TileContext(nc) as tc, ExitStack() as ctx:
        const = ctx.enter_context(tc.tile_pool(name="const", bufs=1))
        dram = ctx.enter_context(tc.tile_pool(name="dram", bufs=1, space="DRAM"))

        # ---- latency-ordered loads (sync queue, dep-chained) -------------
        ld = []
        wpg_sb = const.tile([128, 4 * 128], BF16)
        ld.append(nc.sync.dma_start(
            wpg_sb, bass.AP(tensor=wpg.tensor, offset=0,
                            ap=[[128, 128], [128 * 128, 4], [1, 128]])))
        xt = const.tile([128, 4 * 2 * BLK], BF16)    # (IN-chunk, [prev|own] t)
        for h in (1, 0):                             # own half first
            ld.append(nc.sync.dma_start(
                _ap(xt, h * BLK, [[2 * BLK, 4], [1, BLK]]),
                bass.AP(tensor=xT_in.tensor, offset=h * BLK,
                        ap=[[2 * BLK, 128], [128 * 2 * BLK, 4], [1, BLK]])))
        trig_sb = const.tile([128, 2 * BLK], BF16)
        tld = nc.sync.dma_start(trig_sb, trig)
        ld.insert(2, tld)    # trig may land before xt-prev
        cosb = trig_sb[:, 0:BLK]
        sinb = trig_sb[:, BLK:2 * BLK]
        mcol_sb = const.tile([128, M_NCOL], FP32)
        ld.append(nc.sync.dma_start(mcol_sb, mcol))
        rpv_sb = const.tile([128, NTILE * BLK], BF16)
        ld.append(nc.sync.dma_start(rpv_sb, rpv))
        ones_sb = const.tile([1, 128], BF16)
        ld.append(nc.sync.dma_start(ones_sb, ones_row))
        brow_sb = const.tile([1, 3 * OUT], BF16)
        ld.append(nc.sync.dma_start(brow_sb, brow))
        wgs_sb = const.tile([128, 8 * OUT], BF16)
        ld.append(nc.sync.dma_start(
            wgs_sb, bass.AP(tensor=wgs.tensor, offset=0,
                            ap=[[OUT, 128], [128 * OUT, 8], [1, OUT]])))
        _chain(ld)
        rpw_sb = const.tile([128, NTILE * BLK], BF16)
        wmix_sb = const.tile([128, NTILE * 2 * OUT], BF16)

        def rprev(t):
            return rpv_sb[:, t * BLK:(t + 1) * BLK]

        def rpow(t):
            return rpw_sb[:, t * BLK:(t + 1) * BLK]

        eps_sb = const.tile([128, 1], FP32)
        nc.vector.memset(eps_sb, LN_EPS)
        # pre-warm the ACT function tables off the critical path
        actw = const.tile([1, 2], FP32)
        nc.vector.memset(actw, 1.0)
        for fn in (AFT.Identity, AFT.Sigmoid, AFT.Sqrt):
            nc.scalar.activation(actw, actw, fn)

        zD = dram.tile([TR, 2 * BLK], BF16, name="zD")
        E_my = dram.tile([128, 2 * NTILE], FP32, name="E_my")
        E_all = dram.tile([128 * NCORES, 2 * NTILE], FP32, name="E_all")

        # ---- PE warmup: p-state ramp on dummy matmuls --------------------
        warm = const.tile([128, BLK], BF16)
        nc.vector.memset(warm, 0.0)
        with tc.tile_pool(name="psw", bufs=1, space="PSUM") as psw:
            wps = psw.tile([128, BLK], FP32, tag="warm")
            for i in range(3):
                nc.tensor.matmul(wps, warm[:, 0:128], warm,
                                 start=(i == 0), stop=(i == 2))

        # ---- A: gated z, own block first (feeds the scan/E chain) --------
        zb_all = const.tile([128, NTILE * BLK], BF16)
        zbs = [zb_all[:, t * BLK:(t + 1) * BLK] for t in range(NTILE)]
        with tc.tile_pool(name="psa", bufs=2, space="PSUM") as psa:
            for h in (1, 0):
                ps = psa.tile([128, BLK], FP32, tag="za", bufs=2)
                for ki in range(4):
                    nc.tensor.matmul(
                        ps, wpg_sb[:, ki * 128:(ki + 1) * 128],
                        xt[:, ki * 2 * BLK + h * BLK: ki * 2 * BLK + (h + 1) * BLK],
                        start=(ki == 0), stop=(ki == 3))
                pre_sb = const.tile([64, BLK], FP32, tag=f"pre{h}")
                nc.scalar.activation(pre_sb, ps[0:64, :], AFT.Identity,
                                     bias=mcol_sb[0:64, M_BIAS:M_BIAS + 1])
                sig_sb = const.tile([64, BLK], FP32, tag=f"sig{h}")
                nc.scalar.activation(sig_sb, ps[64:128, :], AFT.Sigmoid,
                                     bias=mcol_sb[64:128, M_BIAS:M_BIAS + 1])
                zt = const.tile([64, BLK], BF16, tag=f"z{h}")
                if h == 0:   # prev block: masked to 0 on core 0
                    nc.vector.scalar_tensor_tensor(
                        zt, pre_sb, mcol_sb[0:64, M_MASK:M_MASK + 1], sig_sb,
                        op0=AOT.mult, op1=AOT.mult)
                else:
                    nc.vector.tensor_mul(zt, pre_sb, sig_sb)
                nc.sync.dma_start(
                    bass.AP(tensor=zD.tensor, offset=zD.offset + h * BLK,
                            ap=[[2 * BLK, TR], [1, BLK]]), zt)
                if h == 1:   # broadcast own-block z for the scan traces now
                    for il in range(2):
                        base = zb_all[il * CTX:(il + 1) * CTX, :]
                        nc.sync.dma_start(
                            bass.AP(tensor=base.tensor, offset=base.offset,
                                    ap=[base.ap[0], [BLK, NTILE], [1, BLK]]),
                            bass.AP(tensor=zD.tensor,
                                    offset=zD.offset + il * 2 * BLK + BLK,
                                    ap=[[0, CTX], [2 * 2 * BLK, NTILE],
                                        [1, BLK]]))

        # ---- scan tiles: inputs + E columns first, then the scans --------
        cc_ts, ss_ts, C_ts, S_ts = [], [], [], []
        E_sb = const.tile([128, 2 * NTILE], FP32)
        for t in range(NTILE):
            cc_t = const.tile([128, BLK], BF16, tag=f"cc{t}")
            nc.vector.tensor_mul(cc_t, zbs[t], cosb)
            ss_t = const.tile([128, BLK], BF16, tag=f"ss{t}")
            nc.vector.tensor_mul(ss_t, zbs[t], sinb)
            scr = const.tile([128, BLK], BF16, tag="scr", bufs=2)
            nc.vector.scalar_tensor_tensor(
                scr, rprev(t), 1.0, cc_t, op0=AOT.mult, op1=AOT.mult,
                accum_out=E_sb[:, 2 * t:2 * t + 1])
            scr2 = const.tile([128, BLK], BF16, tag="scr", bufs=2)
            nc.vector.scalar_tensor_tensor(
                scr2, rprev(t), 1.0, ss_t, op0=AOT.mult, op1=AOT.mult,
                accum_out=E_sb[:, 2 * t + 1:2 * t + 2])
            cc_ts.append(cc_t)
            ss_ts.append(ss_t)

        # E exchange entirely on the gpsimd queue (no head-of-line blocking)
        nc.gpsimd.dma_start(E_my, E_sb)
        nc.gpsimd.collective_compute(
            "AllGather", AOT.bypass, replica_groups=groups,
            ins=[E_my.opt()], outs=[E_all.opt()])
        E_all_sb = const.tile([128, 16 * NTILE], FP32)
        nc.gpsimd.dma_start(
            E_all_sb,
            bass.AP(tensor=E_all.tensor, offset=E_all.offset,
                    ap=[[2 * NTILE, 128], [256 * NTILE, NCORES],
                        [1, 2 * NTILE]]))

        for t in range(NTILE):
            C_t = const.tile([128, BLK], BF16, tag=f"C{t}")
            nc.vector.tensor_tensor_scan(
                C_t, _free_bcast(mcol_sb[:, M_RHO + t:M_RHO + t + 1], BLK),
                cc_ts[t], initial=0.0, op0=AOT.mult, op1=AOT.add)
            S_t = const.tile([128, BLK], BF16, tag=f"S{t}")
            nc.vector.tensor_tensor_scan(
                S_t, _free_bcast(mcol_sb[:, M_RHO + t:M_RHO + t + 1], BLK),
                ss_ts[t], initial=0.0, op0=AOT.mult, op1=AOT.add)
            C_ts.append(C_t)
            S_ts.append(S_t)

        # ---- im2col + G table, interleaved on the ACT HWDGE queue --------
        imcol = const.tile([128, NCH * BLK], BF16)
        g_sb = const.tile([128, NCH * OUT], BF16)
        nq = (NCH + 3) // 4
        gq = [0, nq, 2 * nq, 3 * nq, NCH]
        stream = []

        def load_g(q):
            h0, nh = gq[q], gq[q + 1] - gq[q]
            stream.append(nc.scalar.dma_start(
                _ap(g_sb, h0 * OUT, [[OUT, nh], [1, OUT]]),
                bass.AP(tensor=gtab.tensor, offset=h0 * 128 * OUT,
                        ap=[[OUT, 128], [128 * OUT, nh], [1, OUT]])))

        gq_next = 0

        def maybe_g(c_done):
            nonlocal gq_next
            while gq_next < 4 and gq[gq_next] <= c_done:
                load_g(gq_next)
                gq_next += 1

        maybe_g(0)
        stream.append(nc.scalar.dma_start(rpw_sb, rpw))
        stream.append(nc.scalar.dma_start(
            wmix_sb, bass.AP(tensor=wmix_sc.tensor, offset=0,
                             ap=[[OUT, 128], [128 * OUT, NTILE * 2],
                                 [1, OUT]])))
        for e in CONV_ENTRIES:
            if e[0] == "per_trace":
                _, i, L, c0, k = e
                stream.append(nc.scalar.dma_start(
                    _ap(imcol, c0 * BLK, [[BLK, k], [1, BLK]]),
                    bass.AP(tensor=zD.tensor,
                            offset=zD.offset + i * 2 * BLK + BLK + 1 - L,
                            ap=[[k, 128], [1, k], [1, BLK]])))
                maybe_g(c0 + k)
            else:
                _, i0, nt, L, c0, nch = e
                tpc = 128 // L
                for h in range(tpc):
                    nch_h = (nt - h + tpc - 1) // tpc
                    base = imcol[h * L:(h + 1) * L, :]
                    stream.append(nc.scalar.dma_start(
                        bass.AP(tensor=base.tensor,
                                offset=base.offset + c0 * BLK,
                                ap=[base.ap[0], [BLK, nch_h], [1, BLK]]),
                        bass.AP(tensor=zD.tensor,
                                offset=(zD.offset + (i0 + h) * 2 * BLK
                                        + BLK + 1 - L),
                                ap=[[1, L], [tpc * 2 * BLK, nch_h], [1, BLK]])))
                npad = nch * 128 - ((nt - 1) // tpc) * 128 - \
                    ((nt - 1) % tpc + 1) * L
                if npad > 0:   # ragged tail: fill with dup rows (G=0)
                    base = imcol[128 - npad:128, :]
                    stream.append(nc.scalar.dma_start(
                        bass.AP(tensor=base.tensor,
                                offset=base.offset + (c0 + nch - 1) * BLK,
                                ap=[base.ap[0], [1, BLK]]),
                        bass.AP(tensor=zD.tensor,
                                offset=zD.offset + (TR - 1) * 2 * BLK + BLK,
                                ap=[[0, npad], [1, BLK]])))
                maybe_g(c0 + nch)
        _chain([ld[-1]] + stream)

        # ---- carry correction + rotate-back (DVE, after AllGather) -------
        prod = const.tile([128, 16 * NTILE], FP32)
        nc.vector.tensor_mul(prod, E_all_sb, mcol_sb[:, M_WGT:M_WGT + 16 * NTILE])
        w8 = 8 * NTILE
        f1 = const.tile([128, w8], FP32)
        nc.vector.tensor_add(f1, prod[:, 0:w8], prod[:, w8:2 * w8])
        f2 = const.tile([128, w8 // 2], FP32)
        nc.vector.tensor_add(f2, f1[:, 0:w8 // 2], f1[:, w8 // 2:w8])
        icis = const.tile([128, 2 * NTILE], FP32)
        if with_state0:
            f3 = const.tile([128, 2 * NTILE], FP32)
            nc.vector.tensor_add(f3, f2[:, 0:2 * NTILE], f2[:, 2 * NTILE:])
            nc.vector.tensor_add(icis, f3, mcol_sb[:, M_S0:M_S0 + 2 * NTILE])
        else:
            nc.vector.tensor_add(icis, f2[:, 0:2 * NTILE], f2[:, 2 * NTILE:])

        s_rs, s_is = [], []
        for t in range(NTILE):
            Cc = const.tile([128, BLK], BF16, tag=f"Cc{t}")
            nc.vector.scalar_tensor_tensor(
                Cc, rpow(t), icis[:, 2 * t:2 * t + 1], C_ts[t],
                op0=AOT.mult, op1=AOT.add)
            Sc = const.tile([128, BLK], BF16, tag=f"Sc{t}")
            nc.vector.scalar_tensor_tensor(
                Sc, rpow(t), icis[:, 2 * t + 1:2 * t + 2], S_ts[t],
                op0=AOT.mult, op1=AOT.add)
            m1 = const.tile([128, BLK], BF16, tag="m1", bufs=2)
            nc.vector.tensor_mul(m1, Cc, cosb)
            m2 = const.tile([128, BLK], BF16, tag="m2", bufs=2)
            nc.vector.tensor_mul(m2, Sc, sinb)
            s_r = const.tile([128, BLK], BF16, tag=f"sr{t}")
            nc.vector.tensor_add(s_r, m1, m2)
            m3 = const.tile([128, BLK], BF16, tag="m3", bufs=2)
            nc.vector.tensor_mul(m3, Cc, sinb)
            m4 = const.tile([128, BLK], BF16, tag="m4", bufs=2)
            nc.vector.tensor_mul(m4, Sc, cosb)
            s_i = const.tile([128, BLK], BF16, tag=f"si{t}")
            nc.vector.tensor_sub(s_i, m3, m4)
            s_rs.append(s_r)
            s_is.append(s_i)

        # ---- PE: conv sweeps + B-prep; stops + LayerNorm per psum --------
        gout_st = const.tile([128, 4 * OUT], BF16)
        skip_st = const.tile([128, 4 * OUT], BF16)
        t2_st = const.tile([128, 4 * OUT], BF16)

        with tc.tile_pool(name="psz", bufs=1, space="PSUM") as psz, \
                tc.tile_pool(name="psb", bufs=2, space="PSUM") as psb, \
                tc.tile_pool(name="pb", bufs=2) as pb:
            zmps = [psz.tile([128, OUT], FP32, tag=f"zm{i}", name=f"zm{i}")
                    for i in range(4)]

            def bprep(tc4):
                toff = 512 + tc4 * 128
                osl = slice(tc4 * OUT, (tc4 + 1) * OUT)
                ps_go = psb.tile([128, OUT], FP32, tag="go", bufs=2)
                for ki in range(4):
                    nc.tensor.matmul(
                        ps_go,
                        xt[:, ki * 2 * BLK + toff: ki * 2 * BLK + toff + 128],
                        wgs_sb[:, ki * OUT:(ki + 1) * OUT],
                        start=(ki == 0), stop=False)
                nc.tensor.matmul(ps_go, ones_sb, brow_sb[:, 0:OUT],
                                 start=False, stop=True)
                nc.scalar.activation(gout_st[:, osl], ps_go, AFT.Sigmoid)
                ps_sk = psb.tile([128, OUT], FP32, tag="sk", bufs=2)
                for ki in range(4):
                    nc.tensor.matmul(
                        ps_sk,
                        xt[:, ki * 2 * BLK + toff: ki * 2 * BLK + toff + 128],
                        wgs_sb[:, (4 + ki) * OUT:(5 + ki) * OUT],
                        start=(ki == 0), stop=False)
                nc.tensor.matmul(ps_sk, ones_sb, brow_sb[:, OUT:2 * OUT],
                                 start=False, stop=True)
                nc.scalar.copy(skip_st[:, osl], ps_sk)
                # t2 = (gout-1)*skip precomputed off the critical B path
                nc.gpsimd.scalar_tensor_tensor(
                    t2_st[:, osl], gout_st[:, osl], 1.0, skip_st[:, osl],
                    op0=AOT.subtract, op1=AOT.mult)

            def bphase(tc4):
                osl = slice(tc4 * OUT, (tc4 + 1) * OUT)
                v = pb.tile([128, OUT], BF16, tag="v")
                nc.vector.tensor_mul(v, zmps[tc4], gout_st[:, osl])
                stats = pb.tile([128, 6], FP32, tag="stats")
                nc.vector.bn_stats(stats, v)
                mv = pb.tile([128, 2], FP32, tag="mv")
                nc.vector.bn_aggr(mv, stats)
                sd = pb.tile([128, 1], FP32, tag="sd")
                nc.scalar.activation(sd, mv[:, 1:2], AFT.Sqrt, bias=eps_sb)
                rstd = pb.tile([128, 1], FP32, tag="rstd")
                nc.vector.reciprocal(rstd, sd)
                ln = pb.tile([128, OUT], BF16, tag="ln")
                nc.vector.tensor_scalar(
                    ln, v, mv[:, 0:1], rstd, op0=AOT.subtract, op1=AOT.mult)
                res = pb.tile([128, OUT], FP32, tag="res")
                nc.vector.tensor_sub(res, ln, t2_st[:, osl])
                nc.gpsimd.dma_start(outc[tc4 * 128:(tc4 + 1) * 128, :], res)

            # conv sweeps; B-prep fills the DMA-paced first sweep
            for tc4 in range(4):
                for c in range(NCH):
                    nc.tensor.matmul(
                        zmps[tc4],
                        imcol[:, c * BLK + tc4 * 128: c * BLK + tc4 * 128 + 128],
                        g_sb[:, c * OUT:(c + 1) * OUT],
                        start=(c == 0), stop=False)
                    if tc4 == 0 and c in (2, 6, 10, 14):
                        bprep((2, 6, 10, 14).index(c))
            # stops + LayerNorm per psum
            for tc4 in range(4):
                nc.tensor.matmul(zmps[tc4], ones_sb,
                                 brow_sb[:, 2 * OUT:3 * OUT],
                                 start=False, stop=False)
                for t in range(NTILE):
                    nc.tensor.matmul(
                        zmps[tc4], s_rs[t][:, tc4 * 128:(tc4 + 1) * 128],
                        wmix_sb[:, (2 * t) * OUT:(2 * t + 1) * OUT],
                        start=False, stop=False)
                    nc.tensor.matmul(
                        zmps[tc4], s_is[t][:, tc4 * 128:(tc4 + 1) * 128],
                        wmix_sb[:, (2 * t + 1) * OUT:(2 * t + 2) * OUT],
                        start=False, stop=(t == NTILE - 1))
                bphase(tc4)

    nc.compile()
    return nc


def _prep_inputs(inputs):
    x = np.asarray(inputs["x"], np.float32)
    state0 = np.asarray(inputs["state0"], np.float64)
    a = np.abs(np.asarray(inputs["ffa_a"], np.float64))
    b = np.asarray(inputs["ffa_b"], np.float64)
    rho = np.exp(-a)
    W_pre = np.asarray(inputs["W_pre"], np.float32)
    b_pre = np.asarray(inputs["b_pre"], np.float32)
    W_gin = np.asarray(inputs["W_gin"], np.float32)
    b_gin = np.asarray(inputs["b_gin"], np.float32)
    W_gout = np.asarray(inputs["W_gout"], np.float32)
    b_gout = np.asarray(inputs["b_gout"], np.float32)
    W_skip = np.asarray(inputs["W_skip"], np.float32)
    b_skip = np.asarray(inputs["b_skip"], np.float32)
    W_mix = np.asarray(inputs["W_mix"], np.float64)
    b_mix = np.asarray(inputs["b_mix"], np.float32)
    Wm = W_mix.reshape(TR, 2, CTX, OUT)

    bf16 = mybir.dt.np(BF16)

    # G table (same for all cores)
    G = np.zeros((KCONV, OUT), np.float32)
    for i in range(NSCAN, TR):
        rows = np.nonzero(ROWMAP[:, 0] == i)[0]
        if len(rows) == 0:
            continue
        ds = ROWMAP[rows, 1].astype(np.float64)
        ang = np.outer(ds, b)
        G[rows] = ((np.cos(ang) @ Wm[i, 0] + np.sin(ang) @ Wm[i, 1])
                   * (rho[i] ** ds)[:, None]).astype(np.float32)
    G = G.astype(bf16)

    wpg_h = np.concatenate([W_pre, W_gin], axis=1).astype(bf16)   # (512,128)
    wgs_h = np.concatenate([W_gout.reshape(4, 128, OUT),
                            W_skip.reshape(4, 128, OUT)], axis=0) \
        .reshape(8 * 128, OUT).astype(bf16)
    wmix_h = np.stack(
        [Wm[2 * t + il, fld]
         for t in range(NTILE) for fld in range(2) for il in range(2)]) \
        .reshape(NTILE * 2, 128, OUT).reshape(NTILE * 2 * 128, OUT).astype(bf16)
    ones_h = np.ones((1, 128), bf16)
    brow_h = np.concatenate([b_gout, b_skip, b_mix])[None, :].astype(bf16)

    jj = np.tile(np.arange(CTX), 2)                 # j per partition
    tau = np.arange(BLK, dtype=np.float64)
    rp_h = np.zeros((128, 2 * NTILE * BLK))
    for t in range(NTILE):
        ii = np.repeat([2 * t, 2 * t + 1], CTX)
        rp_h[:, 2 * t * BLK:(2 * t + 1) * BLK] = \
            rho[ii][:, None] ** (BLK - 1.0 - tau[None, :])
        rp_h[:, (2 * t + 1) * BLK:(2 * t + 2) * BLK] = \
            rho[ii][:, None] ** (tau[None, :] + 1.0)
    rp_h = rp_h.astype(bf16)

    s0c = state0[0, :, :, 0] + 1j * state0[0, :, :, 1]   # (TR, CTX)
    r_init = np.exp(1j * b)[None, :] * s0c[0:NSCAN]      # R_{-1} per (i,j)
    initC = r_init.real.reshape(NSCAN, CTX)
    initS = (-r_init.imag).reshape(NSCAN, CTX)

    xb = x.astype(bf16)
    in_maps = []
    for c in range(NCORES):
        t0 = c * BLK
        xT_h = np.zeros((IN, 2 * BLK), bf16)
        if c > 0:
            xT_h[:, 0:BLK] = xb[t0 - BLK:t0].T
        xT_h[:, BLK:] = xb[t0:t0 + BLK].T

        tg = (t0 + np.arange(BLK, dtype=np.float64))[None, :]
        ang = b[jj][:, None] * tg                    # (128, BLK)
        trig_h = np.concatenate([np.cos(ang), np.sin(ang)], axis=1).astype(bf16)

        mcol_h = np.zeros((128, M_NCOL), np.float32)
        for t in range(NTILE):
            ii = np.repeat([2 * t, 2 * t + 1], CTX)
            mcol_h[:, M_RHO + t] = rho[ii]
            for bb in range(c):
                w = rho[ii] ** (512.0 * (c - 1 - bb))
                mcol_h[:, M_WGT + bb * 2 * NTILE + 2 * t] = w
                mcol_h[:, M_WGT + bb * 2 * NTILE + 2 * t + 1] = w
            rr = rho[ii] ** (512.0 * c)
            mcol_h[:, M_S0 + 2 * t] = rr * np.concatenate(
                [initC[2 * t], initC[2 * t + 1]])
            mcol_h[:, M_S0 + 2 * t + 1] = rr * np.concatenate(
                [initS[2 * t], initS[2 * t + 1]])
        mcol_h[0:64, M_MASK] = 0.0 if c == 0 else 1.0
        mcol_h[0:64, M_BIAS] = b_pre
        mcol_h[64:128, M_BIAS] = b_gin

        in_maps.append({
            "xT_in": xT_h,
            "wpg": wpg_h,
            "trig": trig_h,
            "rp": rp_h,
            "mcol": mcol_h,
            "wmix_sc": wmix_h,
            "wgs": wgs_h,
            "gtab": G,
            "ones_row": ones_h,
            "brow": brow_h,
        })
    return in_maps


def _assemble(results) -> np.ndarray:
    return np.concatenate(
        [np.asarray(results[c]["outc"]) for c in range(NCORES)], axis=0)


def _get_module(with_state0: bool = False):
    key = f"m{int(with_state0)}"
    if key not in _CACHE:
        _CACHE[key] = _build_module(with_state0)
    return _CACHE[key]


def kernel(**inputs) -> np.ndarray:
    with_s0 = bool(np.any(np.asarray(inputs["state0"])))
    nc = _get_module(with_s0)
    in_maps = _prep_inputs(inputs)
    res = run_bass_kernel_spmd(nc, in_maps, list(range(NCORES)))
    return _assemble(res.results)


if __name__ == "__main__":
    import reference
    inputs = reference.setup_inputs()
    out = kernel(**{k: np.asarray(v) for k, v in inputs.items()})
    print("kernel output", out.shape, out.dtype)


# revision 15
# speedup vs baseline: 1.2220x; 1.2021x over previous
"""FFM layer on 8 Trainium2 NeuronCores — conv-hybrid, T-block sharded.

Each core owns a 512-row block of the sequence and produces its block of
the output directly; the only collective is a 3 KB AllGather of scan
carries (fully overlapped with the conv matmuls).

  zm[t,o] = sum_{i,Delta} z[t-Delta, i] * G[(i,Delta), o]
  G[(i,D),o] = rho_i^D * sum_j cos(b_j D) Wre[i,j,o] + sin(b_j D) Wim[i,j,o]

Traces 6..63 (kernel decays within <= 128 steps) go through this causal-
conv-as-matmul with per-trace truncation; traces 0..5 (slow decay) use
three 128-partition tiles of rotated real-scan pairs (C,S) over the local
block plus a carry correction C' = C + rho^{tau+1} * I_c, where I_c is a
weighted sum of the other cores' block-end columns E. E is computed
directly as a weighted reduction of the scan INPUT (accum_out of a fused
multiply), so the collective launches before the scans even finish.

Schedule: PE warms up on dummy matmuls (p-state ramp), sync queue carries
the latency-critical loads in dependency-chained order, the ACT HWDGE
queue streams im2col (diagonal DRAM->SBUF views of z) interleaved with
the G table, and each psum's LayerNorm runs right after its stop matmul.
"""

import numpy as np
from contextlib import ExitStack

import concourse.bacc as bacc
import concourse.bass as bass
import concourse.tile as tile
from concourse import mybir
from concourse.bass_utils import run_bass_kernel_spmd

T, IN, TR, CTX, OUT = 4096, 512, 64, 64, 512
NCORES = 8
BLK = T // NCORES       # 512 rows per core
NTILE = 3               # scan tiles (2 traces each)
NSCAN = 2 * NTILE       # traces handled by scan
LN_EPS = 1e-6
FP32 = mybir.dt.float32
BF16 = mybir.dt.bfloat16
AOT = mybir.AluOpType
AFT = mybir.ActivationFunctionType

# conv plan: per-trace entries (trace, L) with L multiple of 128, then
# packed classes (first_trace, n_traces, L) with 128//L traces per chunk.
PER_TRACE = [(6, 128), (7, 128), (8, 128), (9, 128)]
PACKED = [(10, 12, 64), (22, 24, 32), (46, 18, 16)]

# mcol column layout
M_RHO = 0                       # NTILE cols: rho per tile
M_WGT = NTILE                   # 16: tile-0 carry weights (b, fld)
M_S0 = M_WGT + 16               # 2*NTILE: state0 carry term
M_MASK = M_S0 + 2 * NTILE       # 1: prev-block mask
M_BIAS = M_MASK + 1             # 1: b_pre|b_gin
M_NCOL = M_BIAS + 1

_CACHE: dict = {}


def _conv_plan():
    """entries for DMA generation + flat row map [(trace, delta)], -1=pad."""
    entries = []
    rowmap = []
    c0 = 0
    for i, L in PER_TRACE:
        k = L // 128
        entries.append(("per_trace", i, L, c0, k))
        block = np.full((k * 128, 2), (-1, 0), np.int64)
        for cc in range(k):
            for p in range(128):
                dp = p * k + cc
                block[cc * 128 + p] = (i, L - 1 - dp)
        rowmap.append(block)
        c0 += k
    for i0, nt, L in PACKED:
        tpc = 128 // L
        nch = (nt + tpc - 1) // tpc
        entries.append(("packed", i0, nt, L, c0, nch))
        block = np.full((nch * 128, 2), (-1, 0), np.int64)
        for cc in range(nch):
            for h in range(tpc):
                tr = i0 + tpc * cc + h
                if tr >= i0 + nt:
                    continue
                for dpr in range(L):
                    block[cc * 128 + h * L + dpr] = (tr, L - 1 - dpr)
        rowmap.append(block)
        c0 += nch
    return entries, np.concatenate(rowmap), c0


CONV_ENTRIES, ROWMAP, NCH = _conv_plan()
KCONV = NCH * 128


def _ap(t: bass.AP, col_off: int, dims) -> bass.AP:
    """AP over an SBUF tile slice: keep its partition dim, custom free dims."""
    return bass.AP(tensor=t.tensor, offset=t.offset + col_off,
                   ap=[t.ap[0]] + list(dims))


def _free_bcast(col: bass.AP, n: int) -> bass.AP:
    return bass.AP(tensor=col.tensor, offset=col.offset,
                   ap=[col.ap[0], [0, n]])


def _chain(insts):
    """Order-chain a list of instructions (scheduler hint, no semaphores)."""
    for a, b in zip(insts[1:], insts[:-1]):
        if a is not None and b is not None:
            bass._add_dep_helper(a.ins, b.ins, False, "dma priority chain")


def _build_module(with_state0: bool = False):
    nc = bacc.Bacc("TRN2", target_bir_lowering=False, debug=False,
                   num_devices=NCORES)

    def inp(name, shape, dt):
        return nc.dram_tensor(name, list(shape), dt, kind="ExternalInput").ap()

    xT_in = inp("xT_in", (IN, 2 * BLK), BF16)        # [prev block | own block]^T
    wpg = inp("wpg", (IN, 128), BF16)                # [W_pre | W_gin] columns
    trig = inp("trig", (128, 2 * BLK), BF16)         # cos | sin (global t)
    crs0 = inp("crs0", (128, 2 * BLK), BF16)         # rhoprev*cos|sin, tile0 own
    crsp = inp("crsp", (128, 4 * BLK), BF16)         # same, tiles 1-2 prev block
    rpw = inp("rpw", (128, NTILE * BLK), BF16)       # rhopow per tile
    mcol = inp("mcol", (128, M_NCOL), FP32)
    wmix_sc = inp("wmix_sc", (NTILE * 2 * 128, OUT), BF16)
    wgs = inp("wgs", (8 * 128, OUT), BF16)           # gout 4 chunks | skip 4
    gtab = inp("gtab", (KCONV, OUT), BF16)           # conv kernel table
    ones_row = inp("ones_row", (1, 128), BF16)
    brow = inp("brow", (1, 3 * OUT), BF16)           # bgout | bskip | bmix

    outc = nc.dram_tensor("outc", [BLK, OUT], FP32, kind="ExternalOutput").ap()
    groups = [list(range(NCORES))]

    with tile.TileContext(nc) as tc, ExitStack() as ctx:
        const = ctx.enter_context(tc.tile_pool(name="const", bufs=1))
        dram = ctx.enter_context(tc.tile_pool(name="dram", bufs=1, space="DRAM"))

        # ---- latency-ordered loads (sync queue, dep-chained) -------------
        ld = []
        wpg_sb = const.tile([128, 4 * 128], BF16)
        ld.append(nc.sync.dma_start(
            wpg_sb, bass.AP(tensor=wpg.tensor, offset=0,
                            ap=[[128, 128], [128 * 128, 4], [1, 128]])))
        xt = const.tile([128, 4 * 2 * BLK], BF16)    # (IN-chunk, [prev|own] t)
        for h in (1, 0):                             # own half first
            ld.append(nc.sync.dma_start(
                _ap(xt, h * BLK, [[2 * BLK, 4], [1, BLK]]),
                bass.AP(tensor=xT_in.tensor, offset=h * BLK,
                        ap=[[2 * BLK, 128], [128 * 2 * BLK, 4], [1, BLK]])))
        trig_sb = const.tile([128, 2 * BLK], BF16)
        tld = nc.sync.dma_start(trig_sb, trig)
        ld.insert(2, tld)    # trig may land before xt-prev
        cosb = trig_sb[:, 0:BLK]
        sinb = trig_sb[:, BLK:2 * BLK]
        mcol_sb = const.tile([128, M_NCOL], FP32)
        ld.append(nc.sync.dma_start(mcol_sb, mcol))
        crs0_sb = const.tile([128, 2 * BLK], BF16)
        ld.append(nc.sync.dma_start(crs0_sb, crs0))
        crsp_sb = const.tile([128, 4 * BLK], BF16)
        ld.append(nc.sync.dma_start(crsp_sb, crsp))
        ones_sb = const.tile([1, 128], BF16)
        ld.append(nc.sync.dma_start(ones_sb, ones_row))
        brow_sb = const.tile([1, 3 * OUT], BF16)
        ld.append(nc.sync.dma_start(brow_sb, brow))
        wgs_sb = const.tile([128, 8 * OUT], BF16)
        ld.append(nc.sync.dma_start(
            wgs_sb, bass.AP(tensor=wgs.tensor, offset=0,
                            ap=[[OUT, 128], [128 * OUT, 8], [1, OUT]])))
        _chain(ld)
        rpw_sb = const.tile([128, NTILE * BLK], BF16)
        wmix_sb = const.tile([128, NTILE * 2 * OUT], BF16)

        def rpow(t):
            return rpw_sb[:, t * BLK:(t + 1) * BLK]

        eps_sb = const.tile([128, 1], FP32)
        nc.vector.memset(eps_sb, LN_EPS)
        # pre-warm the ACT function tables off the critical path
        actw = const.tile([1, 2], FP32)
        nc.vector.memset(actw, 1.0)
        for fn in (AFT.Sqrt, AFT.Identity, AFT.Sigmoid):
            nc.scalar.activation(actw, actw, fn)

        zD = dram.tile([TR, 2 * BLK], BF16, name="zD")
        E_my = dram.tile([128, 2], FP32, name="E_my")
        E_all = dram.tile([128 * NCORES, 2], FP32, name="E_all")

        # ---- PE warmup: p-state ramp on dummy matmuls --------------------
        warm = const.tile([128, BLK], BF16)
        nc.vector.memset(warm, 0.0)
        with tc.tile_pool(name="psw", bufs=1, space="PSUM") as psw:
            wps = psw.tile([128, BLK], FP32, tag="warm")
            for i in range(3):
                nc.tensor.matmul(wps, warm[:, 0:128], warm,
                                 start=(i == 0), stop=(i == 2))

        # ---- A: gated z, own block first (feeds the scan/E chain) --------
        zb_all = const.tile([128, NTILE * BLK], BF16)
        zbs = [zb_all[:, t * BLK:(t + 1) * BLK] for t in range(NTILE)]
        zbp = const.tile([128, 2 * BLK], BF16)   # prev-block z, tiles 1-2
        with tc.tile_pool(name="psa", bufs=2, space="PSUM") as psa:
            for h in (1, 0):
                ps = psa.tile([128, BLK], FP32, tag="za", bufs=2)
                for ki in range(4):
                    nc.tensor.matmul(
                        ps, wpg_sb[:, ki * 128:(ki + 1) * 128],
                        xt[:, ki * 2 * BLK + h * BLK: ki * 2 * BLK + (h + 1) * BLK],
                        start=(ki == 0), stop=(ki == 3))
                pre_sb = const.tile([64, BLK], FP32, tag=f"pre{h}")
                nc.scalar.activation(pre_sb, ps[0:64, :], AFT.Identity,
                                     bias=mcol_sb[0:64, M_BIAS:M_BIAS + 1])
                sig_sb = const.tile([64, BLK], FP32, tag=f"sig{h}")
                nc.scalar.activation(sig_sb, ps[64:128, :], AFT.Sigmoid,
                                     bias=mcol_sb[64:128, M_BIAS:M_BIAS + 1])
                zt = const.tile([64, BLK], BF16, tag=f"z{h}")
                if h == 0:   # prev block: masked to 0 on core 0
                    nc.vector.scalar_tensor_tensor(
                        zt, pre_sb, mcol_sb[0:64, M_MASK:M_MASK + 1], sig_sb,
                        op0=AOT.mult, op1=AOT.mult)
                else:
                    nc.vector.tensor_mul(zt, pre_sb, sig_sb)
                nc.sync.dma_start(
                    bass.AP(tensor=zD.tensor, offset=zD.offset + h * BLK,
                            ap=[[2 * BLK, TR], [1, BLK]]), zt)
                if h == 1:   # broadcast own-block z for the scan traces now
                    for il in range(2):
                        base = zb_all[il * CTX:(il + 1) * CTX, :]
                        zb_last = nc.sync.dma_start(
                            bass.AP(tensor=base.tensor, offset=base.offset,
                                    ap=[base.ap[0], [BLK, NTILE], [1, BLK]]),
                            bass.AP(tensor=zD.tensor,
                                    offset=zD.offset + il * 2 * BLK + BLK,
                                    ap=[[0, CTX], [2 * 2 * BLK, NTILE],
                                        [1, BLK]]))
                else:        # prev-block z for tiles 1-2 local carries
                    for il in range(2):
                        base = zbp[il * CTX:(il + 1) * CTX, :]
                        nc.sync.dma_start(
                            bass.AP(tensor=base.tensor, offset=base.offset,
                                    ap=[base.ap[0], [BLK, 2], [1, BLK]]),
                            bass.AP(tensor=zD.tensor,
                                    offset=zD.offset + (2 + il) * 2 * BLK,
                                    ap=[[0, CTX], [2 * 2 * BLK, 2],
                                        [1, BLK]]))

        # ---- E columns: tile0 -> collective; tiles 1-2 local (prev z) ----
        E_sb = const.tile([128, 2], FP32)
        scr = const.tile([128, BLK], BF16, tag="scr", bufs=2, name="scr")
        nc.vector.scalar_tensor_tensor(
            scr, crs0_sb[:, 0:BLK], 1.0, zbs[0], op0=AOT.mult, op1=AOT.mult,
            accum_out=E_sb[:, 0:1])
        scr2 = const.tile([128, BLK], BF16, tag="scr", bufs=2, name="scr2")
        nc.vector.scalar_tensor_tensor(
            scr2, crs0_sb[:, BLK:2 * BLK], 1.0, zbs[0],
            op0=AOT.mult, op1=AOT.mult, accum_out=E_sb[:, 1:2])
        # E exchange entirely on the gpsimd queue (no head-of-line blocking)
        nc.gpsimd.dma_start(E_my, E_sb)
        nc.gpsimd.collective_compute(
            "AllGather", AOT.bypass, replica_groups=groups,
            ins=[E_my.opt()], outs=[E_all.opt()])
        E_all_sb = const.tile([128, 16], FP32)
        nc.gpsimd.dma_start(
            E_all_sb,
            bass.AP(tensor=E_all.tensor, offset=E_all.offset,
                    ap=[[2, 128], [256, NCORES], [1, 2]]))

        # local carries for tiles 1-2 (rho^512 <= 3e-4: one-block history)
        ic12 = const.tile([128, 4], FP32)
        for t in (1, 2):
            for f in range(2):
                scc = const.tile([128, BLK], BF16, tag="scr", bufs=2,
                                 name="scc")
                nc.vector.scalar_tensor_tensor(
                    scc, crsp_sb[:, (2 * (t - 1) + f) * BLK:
                                 (2 * (t - 1) + f + 1) * BLK],
                    1.0, zbp[:, (t - 1) * BLK:t * BLK],
                    op0=AOT.mult, op1=AOT.mult,
                    accum_out=ic12[:, 2 * (t - 1) + f:2 * (t - 1) + f + 1])

        # scan inputs
        cc_ts, ss_ts, C_ts, S_ts = [], [], [], []
        for t in range(NTILE):
            cc_t = const.tile([128, BLK], BF16, tag=f"cc{t}", name="cc_t")
            nc.vector.tensor_mul(cc_t, zbs[t], cosb)
            ss_t = const.tile([128, BLK], BF16, tag=f"ss{t}", name="ss_t")
            nc.vector.tensor_mul(ss_t, zbs[t], sinb)
            cc_ts.append(cc_t)
            ss_ts.append(ss_t)

        for t in range(NTILE):
            C_t = const.tile([128, BLK], BF16, tag=f"C{t}")
            nc.vector.tensor_tensor_scan(
                C_t, _free_bcast(mcol_sb[:, M_RHO + t:M_RHO + t + 1], BLK),
                cc_ts[t], initial=0.0, op0=AOT.mult, op1=AOT.add)
            S_t = const.tile([128, BLK], BF16, tag=f"S{t}")
            nc.vector.tensor_tensor_scan(
                S_t, _free_bcast(mcol_sb[:, M_RHO + t:M_RHO + t + 1], BLK),
                ss_ts[t], initial=0.0, op0=AOT.mult, op1=AOT.add)
            C_ts.append(C_t)
            S_ts.append(S_t)

        # ---- im2col + G table, interleaved on the ACT HWDGE queue --------
        imcol = const.tile([128, NCH * BLK], BF16)
        g_sb = const.tile([128, NCH * OUT], BF16)
        nq = (NCH + 3) // 4
        gq = [0, nq, 2 * nq, 3 * nq, NCH]
        stream = []

        def load_g(q):
            h0, nh = gq[q], gq[q + 1] - gq[q]
            stream.append(nc.scalar.dma_start(
                _ap(g_sb, h0 * OUT, [[OUT, nh], [1, OUT]]),
                bass.AP(tensor=gtab.tensor, offset=h0 * 128 * OUT,
                        ap=[[OUT, 128], [128 * OUT, nh], [1, OUT]])))

        gq_next = 0

        def maybe_g(c_done):
            nonlocal gq_next
            while gq_next < 4 and gq[gq_next] <= c_done:
                load_g(gq_next)
                gq_next += 1

        maybe_g(0)
        stream.append(nc.scalar.dma_start(rpw_sb, rpw))
        stream.append(nc.scalar.dma_start(
            wmix_sb, bass.AP(tensor=wmix_sc.tensor, offset=0,
                             ap=[[OUT, 128], [128 * OUT, NTILE * 2],
                                 [1, OUT]])))
        for e in CONV_ENTRIES:
            if e[0] == "per_trace":
                _, i, L, c0, k = e
                stream.append(nc.scalar.dma_start(
                    _ap(imcol, c0 * BLK, [[BLK, k], [1, BLK]]),
                    bass.AP(tensor=zD.tensor,
                            offset=zD.offset + i * 2 * BLK + BLK + 1 - L,
                            ap=[[k, 128], [1, k], [1, BLK]])))
                maybe_g(c0 + k)
            else:
                _, i0, nt, L, c0, nch = e
                tpc = 128 // L
                for h in range(tpc):
                    nch_h = (nt - h + tpc - 1) // tpc
                    base = imcol[h * L:(h + 1) * L, :]
                    stream.append(nc.scalar.dma_start(
                        bass.AP(tensor=base.tensor,
                                offset=base.offset + c0 * BLK,
                                ap=[base.ap[0], [BLK, nch_h], [1, BLK]]),
                        bass.AP(tensor=zD.tensor,
                                offset=(zD.offset + (i0 + h) * 2 * BLK
                                        + BLK + 1 - L),
                                ap=[[1, L], [tpc * 2 * BLK, nch_h], [1, BLK]])))
                npad = nch * 128 - ((nt - 1) // tpc) * 128 - \
                    ((nt - 1) % tpc + 1) * L
                if npad > 0:   # ragged tail: fill with dup rows (G=0)
                    base = imcol[128 - npad:128, :]
                    stream.append(nc.scalar.dma_start(
                        bass.AP(tensor=base.tensor,
                                offset=base.offset + (c0 + nch - 1) * BLK,
                                ap=[base.ap[0], [1, BLK]]),
                        bass.AP(tensor=zD.tensor,
                                offset=zD.offset + (TR - 1) * 2 * BLK + BLK,
                                ap=[[0, npad], [1, BLK]])))
                maybe_g(c0 + nch)
        _chain(stream)
        if stream and zb_last is not None:
            bass._add_dep_helper(stream[0].ins, zb_last.ins, True,
                                 "stream after critical z path")

        # ---- carry correction + rotate-back (tiles 1-2 early, tile0 after
        # the AllGather) -------------------------------------------------
        prod = const.tile([128, 16], FP32)
        nc.vector.tensor_mul(prod, E_all_sb, mcol_sb[:, M_WGT:M_WGT + 16])
        f1 = const.tile([128, 8], FP32)
        nc.vector.tensor_add(f1, prod[:, 0:8], prod[:, 8:16])
        f2 = const.tile([128, 4], FP32)
        nc.vector.tensor_add(f2, f1[:, 0:4], f1[:, 4:8])
        ic0 = const.tile([128, 2], FP32)
        if with_state0:
            f3 = const.tile([128, 2], FP32)
            nc.vector.tensor_add(f3, f2[:, 0:2], f2[:, 2:4])
            nc.vector.tensor_add(ic0, f3, mcol_sb[:, M_S0:M_S0 + 2])
            ic12b = const.tile([128, 4], FP32)
            nc.vector.tensor_add(ic12b, ic12,
                                 mcol_sb[:, M_S0 + 2:M_S0 + 6])
            ic12 = ic12b
        else:
            nc.vector.tensor_add(ic0, f2[:, 0:2], f2[:, 2:4])

        def icol(t, f):
            if t == 0:
                return ic0[:, f:f + 1]
            return ic12[:, 2 * (t - 1) + f:2 * (t - 1) + f + 1]

        s_rs, s_is = [None] * NTILE, [None] * NTILE
        for t in (1, 2, 0):
            Cc = const.tile([128, BLK], BF16, tag=f"Cc{t}", name="Cc")
            nc.vector.scalar_tensor_tensor(
                Cc, rpow(t), icol(t, 0), C_ts[t], op0=AOT.mult, op1=AOT.add)
            Sc = const.tile([128, BLK], BF16, tag=f"Sc{t}", name="Sc")
            nc.vector.scalar_tensor_tensor(
                Sc, rpow(t), icol(t, 1), S_ts[t], op0=AOT.mult, op1=AOT.add)
            m1 = const.tile([128, BLK], BF16, tag="m1", bufs=2, name="m1")
            nc.vector.tensor_mul(m1, Cc, cosb)
            m2 = const.tile([128, BLK], BF16, tag="m2", bufs=2, name="m2")
            nc.vector.tensor_mul(m2, Sc, sinb)
            s_r = const.tile([128, BLK], BF16, tag=f"sr{t}", name="s_r")
            nc.vector.tensor_add(s_r, m1, m2)
            m3 = const.tile([128, BLK], BF16, tag="m3", bufs=2, name="m3")
            nc.vector.tensor_mul(m3, Cc, sinb)
            m4 = const.tile([128, BLK], BF16, tag="m4", bufs=2, name="m4")
            nc.vector.tensor_mul(m4, Sc, cosb)
            s_i = const.tile([128, BLK], BF16, tag=f"si{t}", name="s_i")
            nc.vector.tensor_sub(s_i, m3, m4)
            s_rs[t] = s_r
            s_is[t] = s_i

        # ---- PE: conv sweeps + B-prep; stops + LayerNorm per psum --------
        gout_st = const.tile([128, 4 * OUT], BF16)
        skip_st = const.tile([128, 4 * OUT], BF16)
        t2_st = const.tile([128, 4 * OUT], BF16)

        with tc.tile_pool(name="psz", bufs=1, space="PSUM") as psz, \
                tc.tile_pool(name="psb", bufs=2, space="PSUM") as psb, \
                tc.tile_pool(name="pb", bufs=2) as pb:
            zmps = [psz.tile([128, OUT], FP32, tag=f"zm{i}", name=f"zm{i}")
                    for i in range(4)]

            def bprep(tc4):
                toff = 512 + tc4 * 128
                osl = slice(tc4 * OUT, (tc4 + 1) * OUT)
                ps_go = psb.tile([128, OUT], FP32, tag="go", bufs=2)
                for ki in range(4):
                    nc.tensor.matmul(
                        ps_go,
                        xt[:, ki * 2 * BLK + toff: ki * 2 * BLK + toff + 128],
                        wgs_sb[:, ki * OUT:(ki + 1) * OUT],
                        start=(ki == 0), stop=False)
                nc.tensor.matmul(ps_go, ones_sb, brow_sb[:, 0:OUT],
                                 start=False, stop=True)
                nc.scalar.activation(gout_st[:, osl], ps_go, AFT.Sigmoid)
                ps_sk = psb.tile([128, OUT], FP32, tag="sk", bufs=2)
                for ki in range(4):
                    nc.tensor.matmul(
                        ps_sk,
                        xt[:, ki * 2 * BLK + toff: ki * 2 * BLK + toff + 128],
                        wgs_sb[:, (4 + ki) * OUT:(5 + ki) * OUT],
                        start=(ki == 0), stop=False)
                nc.tensor.matmul(ps_sk, ones_sb, brow_sb[:, OUT:2 * OUT],
                                 start=False, stop=True)
                nc.scalar.copy(skip_st[:, osl], ps_sk)
                # t2 = (gout-1)*skip precomputed off the critical B path
                nc.vector.scalar_tensor_tensor(
                    t2_st[:, osl], gout_st[:, osl], 1.0, skip_st[:, osl],
                    op0=AOT.subtract, op1=AOT.mult)

            # conv sweeps; B-prep fills the DMA-paced first sweep
            for tc4 in range(4):
                for c in range(NCH):
                    nc.tensor.matmul(
                        zmps[tc4],
                        imcol[:, c * BLK + tc4 * 128: c * BLK + tc4 * 128 + 128],
                        g_sb[:, c * OUT:(c + 1) * OUT],
                        start=(c == 0), stop=False)
                    if tc4 == 0 and c in (2, 6, 10, 14):
                        bprep((2, 6, 10, 14).index(c))
            # stops per psum, then stage-pipelined LayerNorm
            for tc4 in range(4):
                nc.tensor.matmul(zmps[tc4], ones_sb,
                                 brow_sb[:, 2 * OUT:3 * OUT],
                                 start=False, stop=False)
                for t in (1, 2, 0):
                    nc.tensor.matmul(
                        zmps[tc4], s_rs[t][:, tc4 * 128:(tc4 + 1) * 128],
                        wmix_sb[:, (2 * t) * OUT:(2 * t + 1) * OUT],
                        start=False, stop=False)
                    nc.tensor.matmul(
                        zmps[tc4], s_is[t][:, tc4 * 128:(tc4 + 1) * 128],
                        wmix_sb[:, (2 * t + 1) * OUT:(2 * t + 2) * OUT],
                        start=False, stop=(t == 0))

            vs, mus, rstds, s1s, s2s = [], [], [], [], []
            for tc4 in range(4):
                osl = slice(tc4 * OUT, (tc4 + 1) * OUT)
                v = pb.tile([128, OUT], BF16, tag=f"v{tc4}", name="v")
                s1 = pb.tile([128, 1], FP32, tag=f"s1{tc4}", name="s1")
                nc.vector.scalar_tensor_tensor(
                    v, zmps[tc4], 1.0, gout_st[:, osl],
                    op0=AOT.mult, op1=AOT.mult, accum_out=s1)
                v2 = pb.tile([128, OUT], BF16, tag="v2", bufs=2, name="v2")
                s2 = pb.tile([128, 1], FP32, tag=f"s2{tc4}", name="s2")
                nc.vector.scalar_tensor_tensor(
                    v2, v, 1.0, v, op0=AOT.mult, op1=AOT.mult, accum_out=s2)
                vs.append(v)
                s1s.append(s1)
                s2s.append(s2)
            for tc4 in range(4):
                mu = pb.tile([128, 1], FP32, tag=f"mu{tc4}", name="mu")
                nc.vector.tensor_scalar(mu, s1s[tc4], 1.0 / OUT, None,
                                        op0=AOT.mult)
                mu2 = pb.tile([128, 1], FP32, tag="mu2", bufs=2, name="mu2")
                nc.vector.tensor_mul(mu2, mu, mu)
                var = pb.tile([128, 1], FP32, tag="var", bufs=2, name="var")
                nc.vector.scalar_tensor_tensor(
                    var, s2s[tc4], 1.0 / OUT, mu2,
                    op0=AOT.mult, op1=AOT.subtract)
                sd = pb.tile([128, 1], FP32, tag="sd", bufs=2, name="sd")
                nc.scalar.activation(sd, var, AFT.Sqrt, bias=eps_sb)
                rstd = pb.tile([128, 1], FP32, tag=f"rstd{tc4}", name="rstd")
                nc.vector.reciprocal(rstd, sd)
                mus.append(mu)
                rstds.append(rstd)
            for tc4 in range(4):
                osl = slice(tc4 * OUT, (tc4 + 1) * OUT)
                ln = pb.tile([128, OUT], BF16, tag="ln", bufs=2, name="ln")
                nc.vector.tensor_scalar(
                    ln, vs[tc4], mus[tc4], rstds[tc4],
                    op0=AOT.subtract, op1=AOT.mult)
                res = pb.tile([128, OUT], FP32, tag="res", bufs=2, name="res")
                nc.vector.tensor_sub(res, ln, t2_st[:, osl])
                nc.gpsimd.dma_start(outc[tc4 * 128:(tc4 + 1) * 128, :], res)

    nc.compile()
    return nc


def _prep_inputs(inputs):
    x = np.asarray(inputs["x"], np.float32)
    state0 = np.asarray(inputs["state0"], np.float64)
    a = np.abs(np.asarray(inputs["ffa_a"], np.float64))
    b = np.asarray(inputs["ffa_b"], np.float64)
    rho = np.exp(-a)
    W_pre = np.asarray(inputs["W_pre"], np.float32)
    b_pre = np.asarray(inputs["b_pre"], np.float32)
    W_gin = np.asarray(inputs["W_gin"], np.float32)
    b_gin = np.asarray(inputs["b_gin"], np.float32)
    W_gout = np.asarray(inputs["W_gout"], np.float32)
    b_gout = np.asarray(inputs["b_gout"], np.float32)
    W_skip = np.asarray(inputs["W_skip"], np.float32)
    b_skip = np.asarray(inputs["b_skip"], np.float32)
    W_mix = np.asarray(inputs["W_mix"], np.float64)
    b_mix = np.asarray(inputs["b_mix"], np.float32)
    Wm = W_mix.reshape(TR, 2, CTX, OUT)

    bf16 = mybir.dt.np(BF16)

    # G table (same for all cores)
    G = np.zeros((KCONV, OUT), np.float32)
    for i in range(NSCAN, TR):
        rows = np.nonzero(ROWMAP[:, 0] == i)[0]
        if len(rows) == 0:
            continue
        ds = ROWMAP[rows, 1].astype(np.float64)
        ang = np.outer(ds, b)
        G[rows] = ((np.cos(ang) @ Wm[i, 0] + np.sin(ang) @ Wm[i, 1])
                   * (rho[i] ** ds)[:, None]).astype(np.float32)
    G = G.astype(bf16)

    wpg_h = np.concatenate([W_pre, W_gin], axis=1).astype(bf16)   # (512,128)
    wgs_h = np.concatenate([W_gout.reshape(4, 128, OUT),
                            W_skip.reshape(4, 128, OUT)], axis=0) \
        .reshape(8 * 128, OUT).astype(bf16)
    wmix_h = np.stack(
        [Wm[2 * t + il, fld]
         for t in range(NTILE) for fld in range(2) for il in range(2)]) \
        .reshape(NTILE * 2, 128, OUT).reshape(NTILE * 2 * 128, OUT).astype(bf16)
    ones_h = np.ones((1, 128), bf16)
    brow_h = np.concatenate([b_gout, b_skip, b_mix])[None, :].astype(bf16)

    jj = np.tile(np.arange(CTX), 2)                 # j per partition
    tau = np.arange(BLK, dtype=np.float64)
    rpw_h = np.zeros((128, NTILE * BLK))
    rrev = np.zeros((128, NTILE))                   # rho^{511-tau} factors
    for t in range(NTILE):
        ii = np.repeat([2 * t, 2 * t + 1], CTX)
        rpw_h[:, t * BLK:(t + 1) * BLK] = \
            rho[ii][:, None] ** (tau[None, :] + 1.0)
    rpw_h = rpw_h.astype(bf16)

    s0c = state0[0, :, :, 0] + 1j * state0[0, :, :, 1]   # (TR, CTX)
    r_init = np.exp(1j * b)[None, :] * s0c[0:NSCAN]      # R_{-1} per (i,j)
    initC = r_init.real.reshape(NSCAN, CTX)
    initS = (-r_init.imag).reshape(NSCAN, CTX)

    xb = x.astype(bf16)
    in_maps = []
    for c in range(NCORES):
        t0 = c * BLK
        xT_h = np.zeros((IN, 2 * BLK), bf16)
        if c > 0:
            xT_h[:, 0:BLK] = xb[t0 - BLK:t0].T
        xT_h[:, BLK:] = xb[t0:t0 + BLK].T

        tg = (t0 + np.arange(BLK, dtype=np.float64))[None, :]
        ang = b[jj][:, None] * tg                    # (128, BLK)
        trig_h = np.concatenate([np.cos(ang), np.sin(ang)], axis=1).astype(bf16)

        # E-reduction tables: rho^{511-tau} * cos/sin(b * global t)
        ii0 = np.repeat([0, 1], CTX)
        rv0 = rho[ii0][:, None] ** (BLK - 1.0 - tau[None, :])
        crs0_h = np.concatenate(
            [rv0 * np.cos(ang), rv0 * np.sin(ang)], axis=1).astype(bf16)
        angp = b[jj][:, None] * (tg - BLK)           # prev block angles
        crsp_h = np.zeros((128, 4 * BLK))
        for t in (1, 2):
            iip = np.repeat([2 * t, 2 * t + 1], CTX)
            rvp = rho[iip][:, None] ** (BLK - 1.0 - tau[None, :])
            crsp_h[:, (2 * (t - 1)) * BLK:(2 * (t - 1) + 1) * BLK] = \
                rvp * np.cos(angp)
            crsp_h[:, (2 * (t - 1) + 1) * BLK:(2 * (t - 1) + 2) * BLK] = \
                rvp * np.sin(angp)
        crsp_h = crsp_h.astype(bf16)

        mcol_h = np.zeros((128, M_NCOL), np.float32)
        for t in range(NTILE):
            ii = np.repeat([2 * t, 2 * t + 1], CTX)
            mcol_h[:, M_RHO + t] = rho[ii]
            rr = rho[ii] ** (512.0 * c)
            mcol_h[:, M_S0 + 2 * t] = rr * np.concatenate(
                [initC[2 * t], initC[2 * t + 1]])
            mcol_h[:, M_S0 + 2 * t + 1] = rr * np.concatenate(
                [initS[2 * t], initS[2 * t + 1]])
        for bb in range(c):
            w = rho[ii0] ** (512.0 * (c - 1 - bb))
            mcol_h[:, M_WGT + 2 * bb] = w
            mcol_h[:, M_WGT + 2 * bb + 1] = w
        mcol_h[0:64, M_MASK] = 0.0 if c == 0 else 1.0
        mcol_h[0:64, M_BIAS] = b_pre
        mcol_h[64:128, M_BIAS] = b_gin

        in_maps.append({
            "xT_in": xT_h,
            "wpg": wpg_h,
            "trig": trig_h,
            "crs0": crs0_h,
            "crsp": crsp_h,
            "rpw": rpw_h,
            "mcol": mcol_h,
            "wmix_sc": wmix_h,
            "wgs": wgs_h,
            "gtab": G,
            "ones_row": ones_h,
            "brow": brow_h,
        })
    return in_maps


def _assemble(results) -> np.ndarray:
    return np.concatenate(
        [np.asarray(results[c]["outc"]) for c in range(NCORES)], axis=0)


def _get_module(with_state0: bool = False):
    key = f"m{int(with_state0)}"
    if key not in _CACHE:
        _CACHE[key] = _build_module(with_state0)
    return _CACHE[key]


def kernel(**inputs) -> np.ndarray:
    with_s0 = bool(np.any(np.asarray(inputs["state0"])))
    nc = _get_module(with_s0)
    in_maps = _prep_inputs(inputs)
    res = run_bass_kernel_spmd(nc, in_maps, list(range(NCORES)))
    return _assemble(res.results)


if __name__ == "__main__":
    import reference
    inputs = reference.setup_inputs()
    out = kernel(**{k: np.asarray(v) for k, v in inputs.items()})
    print("kernel output", out.shape, out.dtype)
